# revision 1
# baseline (speedup 1.0000x reference)
"""Trainium2 Bass kernel for nn_Kmeans (vq_codebook).

Sharding: one head per NeuronCore (h = core id, 8 heads / 8 cores).

Device work per core (head h):
  - dists o1 [l-chunk=128, c=256] via PE matmul (xT chunk stationary, meansT moving)
  - row max (DVE reduce) -> per-position max dist (for the commitment loss)
  - one-hot = is_ge(dists, rowmax) (DVE) -> scatter sums via PE matmul
    (one-hot stationary, x|1 moving) accumulated in PSUM over all l and b
  - dists o2 [c-half=128, l] via PE matmul -> SBUF, then per-256-segment
    top-8 values+indices via DVE max8/max_index -> candidate tables
Host finishes: exact top-32 per cluster from the 8-per-segment candidate
tables (with exact recompute fallback for the astronomically-rare cases the
tables cannot resolve), new_means normalization, and the loss assembled from
exact device partial sums.
"""

import numpy as np

B, NH, L, D, C = 4, 8, 8192, 64, 256
W = 32
NCHUNK = L // 128          # 64 l-chunks per batch entry
NSEG = 32                  # 256-wide segments per half-row
SEG = L // NSEG            # 256
COMMITMENT = 1e-4
EPS = 1e-12

_compiled = None


def _build():
    import concourse.mybir as mybir
    import concourse.tile as tile
    from concourse import bacc

    f32 = mybir.dt.float32
    u32 = mybir.dt.uint32
    X = mybir.AxisListType.X
    OP = mybir.AluOpType

    nc = bacc.Bacc("TRN2", target_bir_lowering=False, debug=False, num_devices=8)

    xT_d = nc.dram_tensor("xT", [B, D, L], f32, kind="ExternalInput")
    xa_d = nc.dram_tensor("xaug", [B, L, D + 1], f32, kind="ExternalInput")
    mT_d = nc.dram_tensor("meansT", [D, C], f32, kind="ExternalInput")

    segv_d = nc.dram_tensor("segv", [B, 2, 128, NSEG * 8], f32, kind="ExternalOutput")
    segi_d = nc.dram_tensor("segi", [B, 2, 128, NSEG * 8], u32, kind="ExternalOutput")
    mx_d = nc.dram_tensor("maxv", [B, 128, NCHUNK], f32, kind="ExternalOutput")
    sums_d = nc.dram_tensor("sums", [2, 128, D + 1], f32, kind="ExternalOutput")

    with tile.TileContext(nc) as tc:
        with (
            tc.tile_pool(name="const", bufs=1) as const_pool,
            tc.tile_pool(name="xt", bufs=2) as xt_pool,
            tc.tile_pool(name="xa", bufs=2) as xa_pool,
            tc.tile_pool(name="mx", bufs=2) as mx_pool,
            tc.tile_pool(name="oh", bufs=3) as oh_pool,
            tc.tile_pool(name="dt", bufs=2) as dt_pool,
            tc.tile_pool(name="seg", bufs=2) as seg_pool,
            tc.tile_pool(name="fin", bufs=1) as fin_pool,
            tc.tile_pool(name="ps_o1", bufs=2, space="PSUM") as ps_o1,
            tc.tile_pool(name="ps_o2", bufs=2, space="PSUM") as ps_o2,
            tc.tile_pool(name="ps_sum", bufs=1, space="PSUM") as ps_sum,
        ):
            mT_sb = const_pool.tile([D, C], f32)
            nc.sync.dma_start(mT_sb[:], mT_d[:])

            sums_ps0 = ps_sum.tile([128, D + 1], f32)
            sums_ps1 = ps_sum.tile([128, D + 1], f32)

            for b in range(B):
                xT_sb = xt_pool.tile([D, L], f32)
                nc.sync.dma_start(xT_sb[:], xT_d[b])
                xa_sb = xa_pool.tile([128, NCHUNK, D + 1], f32)
                nc.sync.dma_start(
                    xa_sb[:], xa_d[b].rearrange("(i p) f -> p i f", p=128)
                )
                mx_sb = mx_pool.tile([128, NCHUNK], f32)

                for i in range(NCHUNK):
                    o1 = ps_o1.tile([128, C], f32)
                    nc.tensor.matmul(
                        o1[:],
                        xT_sb[:, i * 128 : (i + 1) * 128],
                        mT_sb[:],
                        start=True,
                        stop=True,
                    )
                    nc.vector.tensor_reduce(
                        mx_sb[:, i : i + 1], o1[:], axis=X, op=OP.max
                    )
                    oh = oh_pool.tile([128, C], f32)
                    nc.vector.tensor_scalar(
                        oh[:], o1[:], mx_sb[:, i : i + 1], None, op0=OP.is_ge
                    )
                    first = b == 0 and i == 0
                    last = b == B - 1 and i == NCHUNK - 1
                    nc.tensor.matmul(
                        sums_ps0[:],
                        oh[:, 0:128],
                        xa_sb[:, i, :],
                        start=first,
                        stop=last,
                        skip_group_check=True,
                    )
                    nc.tensor.matmul(
                        sums_ps1[:],
                        oh[:, 128:256],
                        xa_sb[:, i, :],
                        start=first,
                        stop=last,
                        skip_group_check=True,
                    )
                nc.sync.dma_start(mx_d[b], mx_sb[:])

                for hf in range(2):
                    dtt = dt_pool.tile([128, L], f32)
                    for w in range(L // 512):
                        o2 = ps_o2.tile([128, 512], f32)
                        nc.tensor.matmul(
                            o2[:],
                            mT_sb[:, hf * 128 : (hf + 1) * 128],
                            xT_sb[:, w * 512 : (w + 1) * 512],
                            start=True,
                            stop=True,
                        )
                        nc.scalar.copy(dtt[:, w * 512 : (w + 1) * 512], o2[:])
                    sv = seg_pool.tile([128, NSEG * 8], f32)
                    si = seg_pool.tile([128, NSEG * 8], u32)
                    for s in range(NSEG):
                        nc.vector.max(
                            out=sv[:, s * 8 : (s + 1) * 8],
                            in_=dtt[:, s * SEG : (s + 1) * SEG],
                        )
                        nc.vector.max_index(
                            si[:, s * 8 : (s + 1) * 8],
                            sv[:, s * 8 : (s + 1) * 8],
                            dtt[:, s * SEG : (s + 1) * SEG],
                        )
                    nc.sync.dma_start(segv_d[b, hf], sv[:])
                    nc.sync.dma_start(segi_d[b, hf], si[:])

            fs0 = fin_pool.tile([128, D + 1], f32)
            fs1 = fin_pool.tile([128, D + 1], f32)
            nc.vector.tensor_copy(fs0[:], sums_ps0[:])
            nc.vector.tensor_copy(fs1[:], sums_ps1[:])
            nc.sync.dma_start(sums_d[0], fs0[:])
            nc.sync.dma_start(sums_d[1], fs1[:])

    nc.compile()
    return nc


def _get_compiled():
    global _compiled
    if _compiled is None:
        _compiled = _build()
    return _compiled


def _run_device(x, means, trace=False):
    from concourse.bass_utils import run_bass_kernel_spmd

    nc = _get_compiled()
    ones = np.ones((B, NH, L, 1), np.float32)
    xaug = np.concatenate([x, ones], axis=-1)
    xT = np.swapaxes(x, 2, 3)
    in_maps = []
    for h in range(NH):
        in_maps.append(
            {
                "xT": np.ascontiguousarray(xT[:, h]),
                "xaug": np.ascontiguousarray(xaug[:, h]),
                "meansT": np.ascontiguousarray(means[h].T),
            }
        )
    res = run_bass_kernel_spmd(nc, in_maps, core_ids=list(range(8)), trace=trace)
    return res


def _finish_host(x, means, outs):
    # ---- top-32 indices per (b, h, cluster) ----
    seg_of_tab = (np.arange(NSEG * 8) // 8) * SEG  # [256] global base of each slot
    indices = np.empty((B, NH, C * W), np.int32)
    n_fallback = 0
    for h in range(NH):
        o = outs[h]
        segv = o["segv"]  # [B, 2, 128, 256] f32
        segi = o["segi"].astype(np.int64)  # local idx within segment
        gidx = segi + seg_of_tab  # [B,2,128,256] global l index
        order = np.argsort(-segv, axis=-1, kind="stable")[..., :W]
        sel = np.take_along_axis(gidx, order, axis=-1)  # [B,2,128,32]

        # fallback triggers: a segment contributing all 8 slots (possible
        # 9th member hidden), or duplicate indices (within-segment f32 tie)
        segsel = np.sort(sel // SEG, axis=-1)
        full_seg = (segsel[..., 7:] == segsel[..., :-7]).any(-1)
        ssel = np.sort(sel, axis=-1)
        dup = (ssel[..., 1:] == ssel[..., :-1]).any(-1)
        bad = full_seg | dup

        for b, hf, p in zip(*np.nonzero(bad)):
            n_fallback += 1
            c = hf * 128 + p
            dd = x[b, h] @ means[h, c]  # [L]
            od = np.argsort(-dd, kind="stable")[:W]
            ssel[b, hf, p] = np.sort(od)

        indices[:, h, :] = ssel.reshape(B, C * W)
    if n_fallback:
        print(f"[kernel] host fallback rows: {n_fallback}")

    # ---- new_means ----
    new_means = np.empty((NH, C, D), np.float32)
    bins_all = np.empty((NH, C), np.float64)
    for h in range(NH):
        s = outs[h]["sums"].reshape(C, D + 1).astype(np.float64)
        bins = s[:, D]
        vec = s[:, :D]
        nrm = np.sqrt((vec * vec).sum(-1, keepdims=True))
        nm = vec / np.maximum(nrm, EPS)
        nm = np.where((bins == 0)[:, None], means[h].astype(np.float64), nm)
        new_means[h] = nm.astype(np.float32)
        bins_all[h] = bins

    # ---- loss ----
    sum_sq_x = float((x.astype(np.float64) ** 2).sum())
    sum_max = 0.0
    for h in range(NH):
        sum_max += float(outs[h]["maxv"].astype(np.float64).sum())
    m2 = (means.astype(np.float64) ** 2).sum(-1)  # [NH, C]
    sum_routed = float((bins_all * m2).sum())
    n = B * NH * L * D
    loss = np.float32((sum_sq_x - 2.0 * sum_max + sum_routed) / n * COMMITMENT)

    return indices, loss, new_means


def kernel(x, means, window_size):
    x = np.asarray(x, dtype=np.float32)
    means = np.asarray(means, dtype=np.float32)
    assert int(window_size) == W
    res = _run_device(x, means)
    return _finish_host(x, means, res.results)
